# revision 7
# baseline (speedup 1.0000x reference)
"""Expected Calibration Error kernel for 8 Trainium2 NeuronCores.

Strategy (data-parallel over rows, per the sharding hint):
  - Pad N=1,000,000 rows to 1,007,616 = 8 * 123 * 1024 with all-zero logit
    rows (label 55); each core processes 123 tiles of 1024 rows.
  - Per tile [128p x (8 rows * 100 cls)]:
      m    = segmented max(logits)                 (DVE tensor_reduce, axis=X)
      kmax = segmented max(logits + eta*iota)      (argmax via tiny index tilt)
      e    = exp(logits)                           (ACT, unshifted: |l| < 7)
      S    = segmented sum(e)                      (DVE tensor_reduce)
  - Epilogue (batched over the 984 staged columns per core):
      conf = exp(m) * (1/S);  idx = round((kmax - m)/eta);  acc = idx == label
      bin  = floor(15*conf)  (== ceil(15*conf)-1 a.e., conf<1 strictly)
      one-hot(bin) -> PE matmuls accumulate per-bin (count, conf_sum, acc_sum)
      into one PSUM [120, 24] (8 column-groups of 15x3 on the block diagonal).
  - Host: extract diagonal blocks, sum over cores/groups, subtract the pad
    rows' contribution (they land in bin 0 deterministically), finish ECE.
"""

import sys

sys.path.insert(0, "/opt/trn_rl_repo")

import numpy as np

import concourse.bass as bass
import concourse.mybir as mybir
import concourse.tile as tile
from concourse.vector_clock import ScopedClock

F32 = mybir.dt.float32
I32 = mybir.dt.int32

N_BINS = 15
C = 100
ROWS_PER_TILE = 1024
A = 8  # rows per partition per tile
ETA = 2.0 ** -20
N_FULL = 1_000_000
N_CORES = 8
TILES = 123
ROWS_PER_CORE = TILES * ROWS_PER_TILE  # 125952
N_PAD = N_CORES * ROWS_PER_CORE       # 1007616
PAD_LABEL = 55
GROUPS_PER_MM = 8  # staged columns per matmul group (lhsT m = 8*15 = 120)


def _patch_tile_drain():
    """walrus in this container allows only 1 sync wait per instruction; split
    every multi-wait instruction's extra waits onto preceding same-engine
    no-ops, and the TileContext exit drain's waits across a chain of drains."""
    if getattr(tile.TileContext, "_drain_patched", False):
        return

    orig_lower = tile.TileContext._lower_ordered_insts

    def _lower_ordered_insts(self, ordered):
        for insts in ordered.values():
            if not isinstance(insts, list):
                continue
            new = []
            for inst in insts:
                si = getattr(inst, "sync_info", None)
                waits = list(si.on_wait) if si is not None else []
                if len(waits) > 1 and isinstance(inst, mybir.Instruction):
                    si.on_wait = waits[-1:]
                    for j, w in enumerate(waits[:-1]):
                        nop = mybir.InstNoOp(
                            name=f"{inst.name}-ws{j}",
                            sync_info=mybir.SyncInfo(on_wait=[w], on_update=[]),
                            bass_nofuse=True,
                            engine=inst.engine,
                        )
                        new.append(nop)
                new.append(inst)
            insts[:] = new
        return orig_lower(self, ordered)

    tile.TileContext._lower_ordered_insts = _lower_ordered_insts

    def _drain_and_barrier(self, tick_clock, wait_clock):
        drain_inst = self.nc.sync.drain()
        wait_clock.add_sem_waits(
            drain_inst.ins, ScopedClock({None: tick_clock.global_clock})
        )
        si = drain_inst.ins.sync_info
        waits = list(si.on_wait) if si else []
        if len(waits) > 1:
            si.on_wait = waits[:1]
            for i in range(1, len(waits)):
                d2 = self.nc.sync.drain()
                d2.ins.sync_info = type(si)(on_wait=waits[i : i + 1], on_update=[])
        self.nc.all_engine_barrier()
        popped = self.nc._tile_sem_poison_stack.pop()
        assert popped is self._sem_poison
        self.nc.clear_and_free_semaphores(list(self.sems.allocated().values()))
        self.nc.all_engine_barrier()

    tile.TileContext._drain_and_barrier = _drain_and_barrier
    tile.TileContext._drain_patched = True


def build_nc(tiles=TILES):
    _patch_tile_drain()
    cols = tiles * A  # staged per-row columns
    n_groups = cols // GROUPS_PER_MM
    assert cols % GROUPS_PER_MM == 0

    nc = bass.Bass(trn_type="TRN2")
    lg = nc.declare_dram_parameter("lg", [tiles * ROWS_PER_TILE, C], F32, isOutput=False)
    lb = nc.declare_dram_parameter("lb", [tiles * ROWS_PER_TILE, 2], I32, isOutput=False)
    part = nc.declare_dram_parameter("part", [GROUPS_PER_MM * N_BINS, GROUPS_PER_MM * 3], F32, isOutput=True)

    with tile.TileContext(nc) as tc:
        with (
            tc.tile_pool(name="io", bufs=3) as io_pool,
            tc.tile_pool(name="ek", bufs=2) as ek_pool,
            tc.tile_pool(name="stage", bufs=1) as stage,
            tc.tile_pool(name="psum", bufs=1, space="PSUM") as psum_pool,
        ):
            # ---- constants ----
            etaiota_i = stage.tile([128, A * C], I32, tag="etaiota_i")
            nc.gpsimd.iota(etaiota_i[:], pattern=[[1, A * C]], base=0, channel_multiplier=0)
            etaiota = stage.tile([128, A * C], F32, tag="etaiota")
            nc.vector.tensor_copy(etaiota[:], etaiota_i[:])
            nc.vector.tensor_scalar_mul(etaiota[:], etaiota[:], ETA)

            # col base: 100 * (col % A), as f32, per staged column
            colbase_i = stage.tile([128, cols], I32, tag="colbase_i")
            nc.gpsimd.iota(colbase_i[:], pattern=[[0, tiles], [C, A]], base=0, channel_multiplier=0)
            colbase = stage.tile([128, cols], F32, tag="colbase")
            nc.vector.tensor_copy(colbase[:], colbase_i[:])

            # one-hot comparison constants 0..14 tiled GROUPS_PER_MM times
            iota15_i = stage.tile([128, GROUPS_PER_MM * N_BINS], I32, tag="iota15_i")
            nc.gpsimd.iota(iota15_i[:], pattern=[[0, GROUPS_PER_MM], [1, N_BINS]], base=0, channel_multiplier=0)
            iota15 = stage.tile([128, GROUPS_PER_MM * N_BINS], F32, tag="iota15")
            nc.vector.tensor_copy(iota15[:], iota15_i[:])

            # ---- staging buffers ----
            m_all = stage.tile([128, cols], F32, tag="m_all")
            kmax_all = stage.tile([128, cols], F32, tag="kmax_all")
            s_all = stage.tile([128, cols], F32, tag="s_all")
            lab_all = stage.tile([128, cols], I32, tag="lab_all")

            # ---- main loop ----
            for t in range(tiles):
                l_tile = io_pool.tile([128, A * C], F32, tag="l_tile")
                r0 = t * ROWS_PER_TILE
                nc.sync.dma_start(
                    out=l_tile[:],
                    in_=lg[r0 : r0 + ROWS_PER_TILE, :].rearrange(
                        "(p a) c -> p (a c)", p=128
                    ),
                )
                nc.sync.dma_start(
                    out=lab_all[:, t * A : (t + 1) * A],
                    in_=lb[r0 : r0 + ROWS_PER_TILE, :].rearrange(
                        "(p a) x -> p a x", p=128
                    )[:, :, 0],
                )

                l3 = l_tile[:].rearrange("p (a c) -> p a c", c=C)
                nc.vector.tensor_reduce(
                    out=m_all[:, t * A : (t + 1) * A],
                    in_=l3,
                    axis=mybir.AxisListType.X,
                    op=mybir.AluOpType.max,
                )

                k_tile = ek_pool.tile([128, A * C], F32, tag="k_tile")
                nc.vector.tensor_add(k_tile[:], l_tile[:], etaiota[:])
                nc.vector.tensor_reduce(
                    out=kmax_all[:, t * A : (t + 1) * A],
                    in_=k_tile[:].rearrange("p (a c) -> p a c", c=C),
                    axis=mybir.AxisListType.X,
                    op=mybir.AluOpType.max,
                )

                e_tile = ek_pool.tile([128, A * C], F32, tag="e_tile")
                nc.scalar.activation(e_tile[:], l_tile[:], mybir.ActivationFunctionType.Exp)
                nc.vector.tensor_reduce(
                    out=s_all[:, t * A : (t + 1) * A],
                    in_=e_tile[:].rearrange("p (a c) -> p a c", c=C),
                    axis=mybir.AxisListType.X,
                    op=mybir.AluOpType.add,
                )

            # ---- epilogue ----
            em = stage.tile([128, cols], F32, tag="em")
            nc.scalar.activation(em[:], m_all[:], mybir.ActivationFunctionType.Exp)

            sinv = stage.tile([128, cols], F32, tag="sinv")
            nc.vector.reciprocal(sinv[:], s_all[:])

            conf = stage.tile([128, cols], F32, tag="conf")
            nc.vector.tensor_mul(conf[:], em[:], sinv[:])

            # bin = floor(15*conf), exact via the 2^23 magic-number RTN trick:
            # round(y - 0.5) = floor(y) for non-integer y; fp add at 2^23
            # quantizes to integers with round-to-nearest.
            y = stage.tile([128, cols], F32, tag="y")
            nc.vector.tensor_scalar_mul(y[:], conf[:], float(N_BINS))
            # magic = 1.5*2^23 keeps the sum in [2^23, 2^24) where the fp grid
            # is exactly 1.0, so the add rounds (y-0.5) to the nearest integer.
            binv = stage.tile([128, cols], F32, tag="binv")
            nc.vector.tensor_scalar_sub(binv[:], y[:], 0.5)
            nc.vector.tensor_scalar_add(binv[:], binv[:], 12582912.0)
            nc.vector.tensor_scalar_sub(binv[:], binv[:], 12582912.0)

            # acc = (round((kmax - m)/eta) - 100*(col%A)) == label
            r = stage.tile([128, cols], F32, tag="r")
            nc.vector.tensor_sub(r[:], kmax_all[:], m_all[:])
            nc.vector.tensor_scalar_mul(r[:], r[:], 1.0 / ETA)
            labf = stage.tile([128, cols], F32, tag="labf")
            nc.vector.tensor_copy(labf[:], lab_all[:])
            nc.vector.tensor_add(labf[:], labf[:], colbase[:])
            dd = stage.tile([128, cols], F32, tag="dd")
            nc.vector.tensor_sub(dd[:], r[:], labf[:])
            nc.vector.tensor_mul(dd[:], dd[:], dd[:])
            acc = stage.tile([128, cols], F32, tag="acc")
            nc.vector.tensor_scalar(acc[:], dd[:], 0.24, scalar2=None, op0=mybir.AluOpType.is_lt)

            # V: interleaved (1, conf, acc) per staged column
            v_all = stage.tile([128, cols * 3], F32, tag="v_all")
            nc.vector.memset(v_all[:], 1.0)
            v3 = v_all[:].rearrange("p (col three) -> p col three", three=3)
            nc.vector.tensor_copy(v3[:, :, 1], conf[:])
            nc.vector.tensor_copy(v3[:, :, 2], acc[:])

            # one-hot: oh[p, g, j, b] = (binv[p, g*A+j] == b)
            oh = stage.tile([128, cols * N_BINS], F32, tag="oh")
            oh4 = oh[:].rearrange("p (g j b) -> p g j b", j=GROUPS_PER_MM, b=N_BINS)
            bin4 = (
                binv[:]
                .rearrange("p (g j) -> p g j", j=GROUPS_PER_MM)
                .unsqueeze(-1)
                .to_broadcast([128, n_groups, GROUPS_PER_MM, N_BINS])
            )
            io4 = (
                iota15[:]
                .rearrange("p (j b) -> p j b", b=N_BINS)
                .unsqueeze(1)
                .to_broadcast([128, n_groups, GROUPS_PER_MM, N_BINS])
            )
            nc.vector.tensor_tensor(out=oh4, in0=bin4, in1=io4, op=mybir.AluOpType.is_equal)

            # PE: accumulate per-bin partials over all column groups
            psum = psum_pool.tile([GROUPS_PER_MM * N_BINS, GROUPS_PER_MM * 3], F32)
            for g in range(n_groups):
                nc.tensor.matmul(
                    out=psum[:],
                    lhsT=oh[:, g * GROUPS_PER_MM * N_BINS : (g + 1) * GROUPS_PER_MM * N_BINS],
                    rhs=v_all[:, g * GROUPS_PER_MM * 3 : (g + 1) * GROUPS_PER_MM * 3],
                    start=(g == 0),
                    stop=(g == n_groups - 1),
                )

            out_sb = stage.tile([GROUPS_PER_MM * N_BINS, GROUPS_PER_MM * 3], F32, tag="out_sb")
            nc.vector.tensor_copy(out_sb[:], psum[:])
            nc.sync.dma_start(out=part[:], in_=out_sb[:])

    return nc


# ----------------------------------------------------------------------------
# host side
# ----------------------------------------------------------------------------

_RUNNER_CACHE = {}


def _get_runner(tiles=TILES):
    """Build (once) a jitted 8-core shard_map runner for the kernel."""
    key = tiles
    if key in _RUNNER_CACHE:
        return _RUNNER_CACHE[key]

    import jax
    from jax.sharding import Mesh, PartitionSpec
    try:
        from jax.experimental.shard_map import shard_map
    except ImportError:
        from jax.shard_map import shard_map
    from concourse import bass2jax

    nc = build_nc(tiles)
    bass2jax.install_neuronx_cc_hook()

    partition_name = nc.partition_id_tensor.name if nc.partition_id_tensor else None
    in_names = ["lg", "lb"]
    out_names = ["part"]
    out_avals = [jax.core.ShapedArray((GROUPS_PER_MM * N_BINS, GROUPS_PER_MM * 3), np.float32)]
    all_in_names = in_names + out_names + ([partition_name] if partition_name else [])

    def _body(*args):
        operands = list(args)
        if partition_name is not None:
            operands.append(bass2jax.partition_id_tensor())
        outs = bass2jax._bass_exec_p.bind(
            *operands,
            out_avals=tuple(out_avals),
            in_names=tuple(all_in_names),
            out_names=tuple(out_names),
            lowering_input_output_aliases=(),
            sim_require_finite=True,
            sim_require_nnan=True,
            nc=nc,
        )
        return tuple(outs)

    devices = jax.devices()[:N_CORES]
    mesh = Mesh(np.asarray(devices), ("core",))
    n_in = len(in_names) + len(out_avals)
    sharded = jax.jit(
        shard_map(
            _body,
            mesh=mesh,
            in_specs=(PartitionSpec("core"),) * n_in,
            out_specs=(PartitionSpec("core"),) * len(out_names),
            check_rep=False,
        ),
        donate_argnums=(len(in_names),),
        keep_unused=True,
    )
    _RUNNER_CACHE[key] = sharded
    return sharded


def _prep_inputs(logits, labels):
    logits = np.asarray(logits)
    labels = np.asarray(labels)
    n = logits.shape[0]
    assert logits.shape == (N_FULL, C) and n == N_FULL, logits.shape
    lab32 = labels.astype(np.int64).view(np.int32).reshape(n, 2)
    pad = N_PAD - n
    lg = np.concatenate([logits, np.zeros((pad, C), np.float32)], axis=0)
    lbp = np.zeros((pad, 2), np.int32)
    lbp[:, 0] = PAD_LABEL
    lb = np.concatenate([lab32, lbp], axis=0)
    return lg, lb, pad


def _finish(parts, pad):
    """parts: (8, 120, 24) psum dumps -> ECE scalar (f32 [1])."""
    agg = np.zeros((N_BINS, 3), np.float64)
    for c in range(parts.shape[0]):
        for g in range(GROUPS_PER_MM):
            agg += parts[c, g * N_BINS : (g + 1) * N_BINS, g * 3 : (g + 1) * 3]
    counts = agg[:, 0].copy()
    conf_sums = agg[:, 1].copy()
    acc_sums = agg[:, 2].copy()
    # all-zero pad rows: conf = exp(0)/100 -> bin 0, acc = 0 (pred 99 != 55)
    counts[0] -= pad
    conf_sums[0] -= pad * float(np.float32(1.0) / np.float32(100.0))
    n = N_FULL
    prop = counts / n
    denom = np.maximum(counts, 1.0)
    avg_conf = conf_sums / denom
    avg_acc = acc_sums / denom
    per_bin = np.where(counts > 0, np.abs(avg_conf - avg_acc) * prop, 0.0)
    return np.array([per_bin.sum()], dtype=np.float32)


def kernel(logits, labels):
    lg, lb, pad = _prep_inputs(logits, labels)
    runner = _get_runner()
    zeros = np.zeros((N_CORES * GROUPS_PER_MM * N_BINS, GROUPS_PER_MM * 3), np.float32)
    (out,) = runner(lg, lb, zeros)
    parts = np.asarray(out).reshape(N_CORES, GROUPS_PER_MM * N_BINS, GROUPS_PER_MM * 3)
    return _finish(parts, pad)
